# revision 2
# baseline (speedup 1.0000x reference)
"""DFlash draft-model loss/acc kernel for 8 Trainium2 NeuronCores.

Sharding: core c -> (batch b = c//4, query-quarter r = c%4).
Each core computes context features + K/V for its batch (bf16 matmuls,
fp32 accumulation), attention + LM head for its 512 draft rows over the
full vocab, and returns per-row stats (raw rowmax, raw label logit,
scaled sum-exp, rstd). The host computes the weighted CE loss and
accuracy from the stats.
"""
import math
import sys

sys.path.insert(0, "/opt/trn_rl_repo")

import numpy as np
import ml_dtypes

import concourse.bass as bass
import concourse.mybir as mybir
import concourse.tile as tile
from concourse import bacc

BF16 = ml_dtypes.bfloat16
F32 = mybir.dt.float32
BF = mybir.dt.bfloat16
AX = mybir.AxisListType
OP = mybir.AluOpType
ACTF = mybir.ActivationFunctionType

L, B, S, D = 3, 2, 2048, 1024
H, DH = 8, 128
NA, BS = 128, 16
Q = NA * BS            # 2048 draft tokens per batch
V = 32000
MASK_ID = V - 1
GAMMA, EPS = 7.0, 1e-6
NCORES, RPG = 8, 4     # 2 batch groups x 4 row-quarters
QL = Q // RPG          # 512 local draft rows per core
SCH = S // 512         # 4 ctx chunks of 512
KVW = (S + QL) // 128  # 20 kv wrap-blocks (16 ctx + 4 draft)
NEG = -1.0e30
NVCH = (V + 511) // 512            # 63 vocab chunks
VG = 2048                          # vocab staging group (cols)
NVG = (V + VG - 1) // VG           # 16 staging groups


def _wrap(x):
    # [K, N] row-major -> (128, K//128, N): [p, kb, n] = x[kb*128 + p, n]
    K, N = x.shape
    return np.ascontiguousarray(x.reshape(K // 128, 128, N).transpose(1, 0, 2))


def _bfw(x):
    return _wrap(np.asarray(x, np.float32)).astype(BF16)


def _rope_tables(pos):
    # pos: [n] int -> cos/sin [64, n] f32 (row j = dim j angle tables)
    inv = (1.0 / (10000.0 ** (np.arange(64, dtype=np.float32) / 64.0))).astype(np.float32)
    ang = inv[:, None] * pos[None, :].astype(np.float32)
    return np.cos(ang).astype(np.float32), np.sin(ang).astype(np.float32)


def host_prep(inputs):
    """Compute index/label/weight arrays and per-core device inputs."""
    ii = np.asarray(inputs["input_ids"]).astype(np.int64)
    anch = np.asarray(inputs["anchor_positions"]).astype(np.int64)
    hs = np.asarray(inputs["hidden_states"], np.float32)
    lmw = np.asarray(inputs["lm_head_weight"], np.float32)
    nw = np.asarray(inputs["norm_weight"], np.float32)
    fc = np.asarray(inputs["fc_weight"], np.float32)
    emb = np.asarray(inputs["embed_table"], np.float32)
    wq = np.asarray(inputs["wq"], np.float32)
    wk = np.asarray(inputs["wk"], np.float32)
    wv = np.asarray(inputs["wv"], np.float32)
    wo = np.asarray(inputs["wo"], np.float32)

    offs = np.arange(BS, dtype=np.int64)
    pos_flat = (anch[:, :, None] + offs[None, None, :]).reshape(B, Q)
    in_bounds = pos_flat < S
    gidx = np.minimum(pos_flat, S - 1)
    all_tok = np.take_along_axis(ii, gidx, axis=1)
    pos_in_block = np.arange(Q) % BS
    is_anchor = pos_in_block == 0
    draft_ids = np.where(is_anchor[None, :], all_tok, MASK_ID)
    labels = np.where((~is_anchor)[None, :] & in_bounds, all_tok, -100)
    lbl = np.maximum(labels, 0)
    anc_q = anch[:, np.arange(Q) // BS]          # [B, Q] anchor per draft row

    # shared (batch-independent) tensors
    sc_q = 1.0 / math.sqrt(DH)
    shared = {
        "fcT": _bfw(fc.T),                        # [3072 -> D] kxm
        "wqTs": _bfw(wq.T * sc_q),
        "wkT": _bfw(wk.T),
        "wvT": _bfw(wv.T),
        "woT": _bfw(wo.T),
        "lmT": _bfw((lmw * nw[None, :]).T),       # (128, 8, 32000)
        "ident_bf": np.eye(128, dtype=np.float32).astype(BF16),
        "ident_f32": np.eye(128, dtype=np.float32),
    }
    cosc, sinc = _rope_tables(np.arange(S))
    shared["cosc"], shared["sinc"] = cosc, sinc
    qi = np.arange(128)
    shared["dmask"] = np.where((qi[:, None] // BS) == (qi[None, :] // BS),
                               0.0, NEG).astype(np.float32)

    per_core = []
    for c in range(NCORES):
        b, r = c // RPG, c % RPG
        sl = slice(r * QL, (r + 1) * QL)
        hcat = hs[:, b].transpose(1, 0, 2).reshape(S, L * D)   # [S, 3072]
        embT = emb[draft_ids[b]].T                              # [D, Q]
        cosd, sind = _rope_tables(pos_flat[b, sl])
        kv = np.arange(S)
        mb = np.where(kv[None, :] < anc_q[b, sl][:, None], 0.0, NEG).astype(np.float32)
        d = dict(shared)
        d.update({
            "hcatT": _bfw(hcat.T),                              # (128,24,2048)
            "embT": _bfw(embT[:, sl]),                          # (128,8,512)
            "lblT": _bfw((lmw * nw[None, :])[lbl[b, sl]].T),    # (128,8,512)
            "maskb": _wrap(mb),                                 # (128,4,2048) f32
            "cosd": cosd, "sind": sind,                         # [64,512]
        })
        per_core.append(d)

    meta = dict(labels=labels, lbl=lbl, pos_in_block=pos_in_block)
    return per_core, meta


def host_reduce(stats_list, meta):
    """stats_list: per-core [4, 512] f32 rows (M0, labdot, sumexp, rstd)."""
    labels = meta["labels"]
    pib = meta["pos_in_block"]
    decay = np.concatenate([np.zeros(1, np.float32),
                            np.exp(-(np.arange(1, BS, dtype=np.float32) - 1.0) / GAMMA)])
    w_all = decay[pib][None, :] * (labels != -100).astype(np.float32)

    num = 0.0
    den = 0.0
    ncorr = 0
    nvalid = int((labels != -100).sum())
    for c in range(NCORES):
        b, r = c // RPG, c % RPG
        st = stats_list[c]
        m0, labd, sexp, rstd = st[0], st[1], st[2], st[3]
        nll = np.log(sexp) - rstd * labd
        w = w_all[b, r * QL:(r + 1) * QL]
        num += float((w * nll).sum())
        den += float(w.sum())
        valid = labels[b, r * QL:(r + 1) * QL] != -100
        ncorr += int(((labd >= m0) & valid).sum())
    loss = np.float32(num / max(den, 1e-6))
    acc = np.float32(ncorr / max(nvalid, 1))
    return loss, acc


_PROG = None


def _rope(nc, pool, dst, src_ps, cos, sin, n):
    """dst[0:64] = x1*cos - x2*sin ; dst[64:128] = x1*sin + x2*cos.
    src_ps: [128, n] psum f32; cos/sin: [64, n] sbuf f32; dst: [128, n] bf16."""
    t1 = pool.tile([64, n], F32, tag="rope_t1")
    t2 = pool.tile([64, n], F32, tag="rope_t2")
    x1, x2 = src_ps[0:64, :], src_ps[64:128, :]
    nc.vector.tensor_mul(t1[:], x1, cos[:])
    nc.vector.tensor_mul(t2[:], x2, sin[:])
    nc.vector.tensor_sub(dst[0:64, :], t1[:], t2[:])
    nc.vector.tensor_mul(t1[:], x1, sin[:])
    nc.vector.tensor_mul(t2[:], x2, cos[:])
    nc.vector.tensor_add(dst[64:128, :], t1[:], t2[:])


def build_program():
    global _PROG
    if _PROG is not None:
        return _PROG
    import os
    phases = os.environ.get("DFLASH_PHASES", "123")
    lmparts = os.environ.get("DFLASH_LM", "abc")
    nc = bacc.Bacc(None, target_bir_lowering=False, debug=False)
    names = {}
    with tile.TileContext(nc) as tc:
        with tc.tile_pool(name="dram", bufs=1, space="DRAM") as dram:
            def din(name, shape, dt=BF):
                t = dram.tile(shape, dt, kind="ExternalInput", name=name)
                names[name] = t.name
                return t

            hcatT = din("hcatT", [128, 24, 2048])
            fcT = din("fcT", [128, 24, 1024])
            wqTs = din("wqTs", [128, 8, 1024])
            wkT = din("wkT", [128, 8, 1024])
            wvT = din("wvT", [128, 8, 1024])
            woT = din("woT", [128, 8, 1024])
            lmT = din("lmT", [128, 8, V])
            embT = din("embT", [128, 8, QL])
            lblT = din("lblT", [128, 8, QL])
            maskb = din("maskb", [128, 4, 2048], F32)
            cosc = din("cosc", [64, S], F32)
            sinc = din("sinc", [64, S], F32)
            cosd = din("cosd", [64, QL], F32)
            sind = din("sind", [64, QL], F32)
            dmask = din("dmask", [128, 128], F32)
            ident_bf = din("ident_bf", [128, 128])
            ident_f32 = din("ident_f32", [128, 128], F32)

            stats = dram.tile([4, QL], F32, kind="ExternalOutput", name="stats")
            names["stats"] = stats.name

            kT_d = dram.tile([128, 8, S + QL], BF, name="kT_scratch")
            v_d = dram.tile([128, KVW, 1024], BF, name="v_scratch")
            den_d = dram.tile([H, QL], F32, name="den_scratch")
            rstd_d = dram.tile([QL], F32, name="rstd_scratch")

            import contextlib
            with contextlib.ExitStack() as ctx:
                # psum pools shared across phases (<= 8 banks total)
                ps_big = ctx.enter_context(tc.tile_pool(name="ps_big", bufs=4, space="PSUM"))
                ps_tr = ctx.enter_context(tc.tile_pool(name="ps_tr", bufs=2, space="PSUM"))
                ps_row = ctx.enter_context(tc.tile_pool(name="ps_row", bufs=2, space="PSUM"))
                persist = ctx.enter_context(tc.tile_pool(name="persist", bufs=1))

                qTr = persist.tile([128, 8, QL], BF)        # roped q, feature-major
                embT_sb = persist.tile([128, 8, QL], BF)
                hbf = persist.tile([128, 8, QL], BF)
                ones_bf = persist.tile([128, 1], BF)
                ones_f32 = persist.tile([128, 1], F32)
                eps_t = persist.tile([1, 1], F32)
                nc.vector.memset(ones_bf[:], 1.0)
                nc.vector.memset(ones_f32[:], 1.0)
                nc.vector.memset(eps_t[:], EPS)
                nc.sync.dma_start(out=embT_sb[:], in_=embT[:])

                # ---------- phase 1: draft projections + ctx K/V ----------
                if "1" in phases:
                  with tc.tile_pool(name="ph1", bufs=2) as ph1, \
                     tc.tile_pool(name="ph1w", bufs=1) as ph1w:
                    wq_sb = ph1w.tile([128, 8, 1024], BF)
                    wk_sb = ph1w.tile([128, 8, 1024], BF)
                    wv_sb = ph1w.tile([128, 8, 1024], BF)
                    cosd_sb = ph1w.tile([64, QL], F32)
                    sind_sb = ph1w.tile([64, QL], F32)
                    cosc_sb = ph1w.tile([64, S], F32)
                    sinc_sb = ph1w.tile([64, S], F32)
                    nc.sync.dma_start(out=wq_sb[:], in_=wqTs[:])
                    nc.sync.dma_start(out=wk_sb[:], in_=wkT[:])
                    nc.sync.dma_start(out=wv_sb[:], in_=wvT[:])
                    nc.sync.dma_start(out=cosd_sb[:], in_=cosd[:])
                    nc.sync.dma_start(out=sind_sb[:], in_=sind[:])
                    nc.sync.dma_start(out=cosc_sb[:], in_=cosc[:])
                    nc.sync.dma_start(out=sinc_sb[:], in_=sinc[:])

                    # draft q/k (feature-major, roped) and v (token-major)
                    for mb in range(8):
                        qp = ps_big.tile([128, QL], F32, tag="mm")
                        for kb in range(8):
                            nc.tensor.matmul(qp[:], wq_sb[:, kb, mb * 128:(mb + 1) * 128],
                                             embT_sb[:, kb, :], start=kb == 0, stop=kb == 7)
                        _rope(nc, ph1, qTr[:, mb, :], qp, cosd_sb, sind_sb, QL)
                    for mb in range(8):
                        kp = ps_big.tile([128, QL], F32, tag="mm")
                        for kb in range(8):
                            nc.tensor.matmul(kp[:], wk_sb[:, kb, mb * 128:(mb + 1) * 128],
                                             embT_sb[:, kb, :], start=kb == 0, stop=kb == 7)
                        kd_sb = ph1.tile([128, QL], BF, tag="kd")
                        _rope(nc, ph1, kd_sb[:], kp, cosd_sb, sind_sb, QL)
                        nc.sync.dma_start(out=kT_d[:, mb, S:S + QL], in_=kd_sb[:])
                    for sm in range(4):
                        for nn2 in range(2):
                            vp = ps_big.tile([128, 512], F32, tag="mm")
                            for kb in range(8):
                                nc.tensor.matmul(vp[:], embT_sb[:, kb, sm * 128:(sm + 1) * 128],
                                                 wv_sb[:, kb, nn2 * 512:(nn2 + 1) * 512],
                                                 start=kb == 0, stop=kb == 7)
                            vd_sb = ph1.tile([128, 512], BF, tag="vd")
                            nc.vector.tensor_copy(vd_sb[:], vp[:])
                            nc.sync.dma_start(out=v_d[:, 16 + sm, nn2 * 512:(nn2 + 1) * 512],
                                              in_=vd_sb[:])

                    # ctx chunks: ctxT -> kcT (roped) + vc
                    for sc in range(SCH):
                        ssl = slice(sc * 512, (sc + 1) * 512)
                        hc_sb = ph1.tile([128, 24, 512], BF, tag="hcat")
                        nc.sync.dma_start(out=hc_sb[:], in_=hcatT[:, :, ssl])
                        ctx_sb = ph1.tile([128, 8, 512], BF, tag="ctx")
                        for mb in range(8):
                            fcmb = ph1.tile([128, 24, 128], BF, tag="fcmb")
                            nc.sync.dma_start(out=fcmb[:], in_=fcT[:, :, mb * 128:(mb + 1) * 128])
                            cp = ps_big.tile([128, 512], F32, tag="mm")
                            for kb in range(24):
                                nc.tensor.matmul(cp[:], fcmb[:, kb, :],
                                                 hc_sb[:, kb, :], start=kb == 0, stop=kb == 23)
                            nc.vector.tensor_copy(ctx_sb[:, mb, :], cp[:])
                        for mb in range(8):
                            kp = ps_big.tile([128, 512], F32, tag="mm")
                            for kb in range(8):
                                nc.tensor.matmul(kp[:], wk_sb[:, kb, mb * 128:(mb + 1) * 128],
                                                 ctx_sb[:, kb, :], start=kb == 0, stop=kb == 7)
                            kc_sb = ph1.tile([128, 512], BF, tag="kc")
                            _rope(nc, ph1, kc_sb[:], kp, cosc_sb[:, ssl], sinc_sb[:, ssl], 512)
                            nc.sync.dma_start(out=kT_d[:, mb, ssl], in_=kc_sb[:])
                        for sm in range(4):
                            for nn2 in range(2):
                                vp = ps_big.tile([128, 512], F32, tag="mm")
                                for kb in range(8):
                                    nc.tensor.matmul(vp[:], ctx_sb[:, kb, sm * 128:(sm + 1) * 128],
                                                     wv_sb[:, kb, nn2 * 512:(nn2 + 1) * 512],
                                                     start=kb == 0, stop=kb == 7)
                                vc_sb = ph1.tile([128, 512], BF, tag="vc")
                                nc.vector.tensor_copy(vc_sb[:], vp[:])
                                nc.sync.dma_start(out=v_d[:, sc * 4 + sm, nn2 * 512:(nn2 + 1) * 512],
                                                  in_=vc_sb[:])

                # ---------- phase 2: attention ----------
                if "2" in phases:
                  with tc.tile_pool(name="ph2", bufs=2) as ph2, \
                     tc.tile_pool(name="ph2c", bufs=1) as ph2c, \
                     tc.tile_pool(name="ph2p", bufs=3) as ph2p:
                    maskb_sb = ph2c.tile([128, 4, 2048], F32)
                    nc.sync.dma_start(out=maskb_sb[:], in_=maskb[:])
                    dmask_sb = ph2c.tile([128, 128], F32)
                    nc.sync.dma_start(out=dmask_sb[:], in_=dmask[:])
                    idb_sb = ph2c.tile([128, 128], BF)
                    nc.sync.dma_start(out=idb_sb[:], in_=ident_bf[:])
                    wo_sb = ph2c.tile([128, 8, 1024], BF)
                    nc.sync.dma_start(out=wo_sb[:], in_=woT[:])
                    outT = ph2c.tile([128, 8, QL], BF)
                    h_sb = ph2c.tile([128, 8, QL], F32)

                    for h in range(H):
                        kh_sb = ph2.tile([128, S + QL], BF, tag="kh")
                        nc.sync.dma_start(out=kh_sb[:], in_=kT_d[:, h, :])
                        vh_sb = ph2.tile([128, KVW, 128], BF, tag="vh")
                        nc.sync.dma_start(out=vh_sb[:], in_=v_d[:, :, h * 128:(h + 1) * 128])
                        pT = ph2.tile([128, KVW, QL], BF, tag="pT")
                        nc.vector.memset(pT[:, 16:20, :], 0.0)
                        for qt in range(4):
                            qsl = slice(qt * 128, (qt + 1) * 128)
                            for cch in range(SCH):
                                sp = ps_big.tile([128, 512], F32, tag="mm")
                                nc.tensor.matmul(sp[:], qTr[:, h, qsl],
                                                 kh_sb[:, cch * 512:(cch + 1) * 512])
                                sm_sb = ph2p.tile([128, 512], F32, tag="smask")
                                nc.vector.tensor_add(sm_sb[:], sp[:],
                                                     maskb_sb[:, qt, cch * 512:(cch + 1) * 512])
                                pr_sb = ph2p.tile([128, 512], BF, tag="probs")
                                nc.scalar.activation(pr_sb[:], sm_sb[:], ACTF.Exp)
                                for j in range(4):
                                    trp = ps_tr.tile([128, 128], BF, tag="small")
                                    nc.tensor.transpose(trp[:], pr_sb[:, j * 128:(j + 1) * 128],
                                                        idb_sb[:])
                                    nc.vector.tensor_copy(pT[:, cch * 4 + j, qsl], trp[:])
                            # draft block: bidirectional within own 128 range
                            sp = ps_tr.tile([128, 128], F32, tag="small")
                            nc.tensor.matmul(sp[:], qTr[:, h, qsl], kh_sb[:, S + qt * 128:S + (qt + 1) * 128])
                            smd = ph2p.tile([128, 128], F32, tag="smaskd")
                            nc.vector.tensor_add(smd[:], sp[:], dmask_sb[:])
                            prd = ph2p.tile([128, 128], BF, tag="probsd")
                            nc.scalar.activation(prd[:], smd[:], ACTF.Exp)
                            trp = ps_tr.tile([128, 128], BF, tag="small")
                            nc.tensor.transpose(trp[:], prd[:], idb_sb[:])
                            nc.vector.tensor_copy(pT[:, 16 + qt, qsl], trp[:])

                        dp = ps_row.tile([1, QL], F32, tag="row")
                        for cc in range(KVW):
                            nc.tensor.matmul(dp[:], ones_bf[:], pT[:, cc, :],
                                             start=cc == 0, stop=cc == KVW - 1)
                        den_sb = ph2.tile([1, QL], F32, tag="den_sb")
                        nc.vector.reciprocal(den_sb[:], dp[:])
                        nc.sync.dma_start(out=den_d[h, :], in_=den_sb[:])
                        den_bc = ph2.tile([128, QL], F32, tag="den_bc")
                        row = den_d[h:h + 1, :]
                        bc_ap = bass.AP(tensor=row.tensor, offset=row.offset,
                                        ap=[[0, 128]] + list(row.ap)[1:])
                        nc.sync.dma_start(out=den_bc[:], in_=bc_ap)

                        ap_ps = ps_big.tile([128, QL], F32, tag="mm")
                        for cc in range(KVW):
                            nc.tensor.matmul(ap_ps[:], vh_sb[:, cc, :], pT[:, cc, :],
                                             start=cc == 0, stop=cc == KVW - 1)
                        nc.vector.tensor_mul(outT[:, h, :], ap_ps[:], den_bc[:])

                    # wo projection + residual + sumsq/rstd
                    sqp = ps_row.tile([1, QL], F32, tag="row")
                    for mb in range(8):
                        op = ps_big.tile([128, QL], F32, tag="mm")
                        for kb in range(8):
                            nc.tensor.matmul(op[:], wo_sb[:, kb, mb * 128:(mb + 1) * 128],
                                             outT[:, kb, :], start=kb == 0, stop=kb == 7)
                        nc.vector.tensor_add(h_sb[:, mb, :], op[:], embT_sb[:, mb, :])
                        nc.vector.tensor_copy(hbf[:, mb, :], h_sb[:, mb, :])
                        hsq = ph2.tile([128, QL], F32, tag="hsq")
                        nc.vector.tensor_mul(hsq[:], h_sb[:, mb, :], h_sb[:, mb, :])
                        nc.tensor.matmul(sqp[:], ones_f32[:], hsq[:],
                                         start=mb == 0, stop=mb == 7)
                    sq_sb = ph2.tile([1, QL], F32, tag="sq_sb")
                    nc.scalar.activation(sq_sb[:], sqp[:], ACTF.Sqrt,
                                         bias=eps_t[0:1, :], scale=1.0 / D)
                    rstd_sb = ph2.tile([1, QL], F32, tag="rstd_sb")
                    nc.vector.reciprocal(rstd_sb[:], sq_sb[:])
                    nc.sync.dma_start(out=rstd_d[:], in_=rstd_sb[0:1, :])
                    nc.sync.dma_start(out=stats[3:4, :], in_=rstd_sb[:])

                # ---------- phase 3: LM head ----------
                if "3" in phases:
                  with tc.tile_pool(name="ph3", bufs=2) as ph3, \
                     tc.tile_pool(name="ph3c", bufs=1) as ph3c, \
                     tc.tile_pool(name="ph3a", bufs=1) as ph3a:
                    lbl_sb = ph3c.tile([128, 8, QL], BF)
                    nc.sync.dma_start(out=lbl_sb[:], in_=lblT[:])
                    idf_sb = ph3c.tile([128, 128], F32)
                    nc.sync.dma_start(out=idf_sb[:], in_=ident_f32[:])
                    m0 = [ph3a.tile([128, 1], F32, name=f"m0_{qt}") for qt in range(4)]
                    se = [ph3a.tile([128, 1], F32, name=f"se_{qt}") for qt in range(4)]
                    rstd_pt = [ph3a.tile([128, 1], F32, name=f"rstd_pt_{qt}") for qt in range(4)]
                    for qt in range(4):
                        nc.vector.memset(m0[qt][:], -3.0e38)
                        nc.vector.memset(se[qt][:], 0.0)
                        rsl = rstd_d[qt * 128:(qt + 1) * 128]
                        nc.sync.dma_start(out=rstd_pt[qt][:], in_=rsl)

                    for vg in range(NVG):
                        ncol = min(VG, V - vg * VG)
                        lmt = ph3.tile([128, 8, VG], BF, tag="lmt")
                        nc.sync.dma_start(out=lmt[:, :, :ncol], in_=lmT[:, :, vg * VG:vg * VG + ncol])
                        for qt in range(4):
                            qsl = slice(qt * 128, (qt + 1) * 128)
                            nch = (ncol + 511) // 512
                            pss = [ps_big.tile([128, 512], F32, tag="mm", name=f"lp_{vg}_{qt}_{i}")
                                   for i in range(nch)]
                            for kb in range(8):
                                for cc in range(nch):
                                    w = min(512, ncol - cc * 512)
                                    nc.tensor.matmul(pss[cc][:, :w], hbf[:, kb, qsl],
                                                     lmt[:, kb, cc * 512:cc * 512 + w],
                                                     start=kb == 0, stop=kb == 7)
                            for cc in range(nch):
                                w = min(512, ncol - cc * 512)
                                lp = pss[cc]
                                if "a" in lmparts:
                                    cmax = ph3.tile([128, 1], F32, tag="cmax")
                                    nc.vector.tensor_reduce(cmax[:], lp[:, :w], axis=AX.X, op=OP.max)
                                    nc.vector.tensor_max(m0[qt][:], m0[qt][:], cmax[:])
                                if "b" in lmparts:
                                    ej = ph3.tile([128, 512], BF, tag="ej")
                                    csum = ph3.tile([128, 1], F32, tag="csum")
                                    nc.scalar.activation(ej[:, :w], lp[:, :w], ACTF.Exp,
                                                         scale=rstd_pt[qt][:], accum_out=csum[:])
                                    nc.vector.tensor_add(se[qt][:], se[qt][:], csum[:])

                    for qt in range(4):
                        qsl = slice(qt * 128, (qt + 1) * 128)
                        if "c" in lmparts:
                            lpp = ps_tr.tile([128, 128], F32, tag="small")
                            for kb in range(8):
                                nc.tensor.matmul(lpp[:], hbf[:, kb, qsl], lbl_sb[:, kb, qsl],
                                                 start=kb == 0, stop=kb == 7)
                            junk = ph3.tile([128, 128], F32, tag="junk")
                            labd = ph3.tile([128, 1], F32, tag="labd")
                            nc.vector.tensor_mul(junk[:], lpp[:], idf_sb[:])
                            nc.vector.tensor_reduce(labd[:], junk[:], axis=AX.X, op=OP.add)
                            nc.sync.dma_start(out=stats[1:2, qsl], in_=labd[:])
                        if "a" in lmparts:
                            nc.sync.dma_start(out=stats[0:1, qsl], in_=m0[qt][:])
                        if "b" in lmparts:
                            nc.sync.dma_start(out=stats[2:3, qsl], in_=se[qt][:])
    nc.compile()
    _PROG = (nc, names)
    return _PROG


def kernel(**inputs):
    import os
    from concourse.bass_utils import run_bass_kernel_spmd
    nc, names = build_program()
    per_core, meta = host_prep(inputs)
    in_maps = []
    for c in range(NCORES):
        m = {}
        for k, arr in per_core[c].items():
            dt = BF16 if arr.dtype == BF16 else np.float32
            m[names[k]] = np.ascontiguousarray(arr, dtype=dt)
        in_maps.append(m)
    trace = bool(os.environ.get("DFLASH_TRACE"))
    if trace:
        try:
            import ntff_shim
            ntff_shim.install()
        except Exception:
            trace = False
    res = run_bass_kernel_spmd(nc, in_maps, list(range(NCORES)), trace=trace)
    kernel.last_exec_ns = res.exec_time_ns
    kernel.last_trace = getattr(res, "instructions_and_trace", None)
    kernel.last_profile_json = getattr(res, "profile_json", None)
    stats_list = [res.results[c][names["stats"]] for c in range(NCORES)]
    return host_reduce(stats_list, meta)



# revision 5
# speedup vs baseline: 1.5284x; 1.5284x over previous
"""DFlash draft-model loss/acc kernel for 8 Trainium2 NeuronCores.

Sharding: core c -> (batch b = c//4, query-quarter r = c%4).
Each core computes context features + K/V for its batch (bf16 matmuls,
fp32 accumulation), attention + LM head for its 512 draft rows over the
full vocab, and returns per-row stats (max exp, raw label logit,
scaled sum-exp, rstd). The host computes the weighted CE loss and
accuracy from the stats.

Phase 2 computes scores kv-major (no transposes); phase 3 runs the LM
head in fp8e4 DoubleRow mode (2x tensor throughput) with scale factors
SH/SW folded out via the exp scale and host-side rescaling.
"""
import math
import sys

sys.path.insert(0, "/opt/trn_rl_repo")

import numpy as np
import ml_dtypes

import concourse.bass as bass
import concourse.mybir as mybir
import concourse.tile as tile
from concourse import bacc

BF16 = ml_dtypes.bfloat16
F8NP = ml_dtypes.float8_e4m3
F32 = mybir.dt.float32
BF = mybir.dt.bfloat16
F8 = mybir.dt.float8e4
AX = mybir.AxisListType
OP = mybir.AluOpType
ACTF = mybir.ActivationFunctionType
DR = mybir.MatmulPerfMode.DoubleRow

L, B, S, D = 3, 2, 2048, 1024
H, DH = 8, 128
NA, BS = 128, 16
Q = NA * BS            # 2048 draft tokens per batch
V = 32000
MASK_ID = V - 1
GAMMA, EPS = 7.0, 1e-6
NCORES, RPG = 8, 4     # 2 batch groups x 4 row-quarters
QL = Q // RPG          # 512 local draft rows per core
SCH = S // 512         # 4 ctx chunks of 512
KVW = (S + QL) // 128  # 20 kv wrap-blocks (16 ctx + 4 draft)
NEG = -1.0e30
VG = 2048                          # vocab staging group (cols)
NVG = (V + VG - 1) // VG           # 16 staging groups
SH = 64.0                          # fp8 scale for hidden states
SW = 2048.0                        # fp8 scale for lm head weights
SHW = SH * SW


def _wrap(x):
    # [K, N] row-major -> (128, K//128, N): [p, kb, n] = x[kb*128 + p, n]
    K, N = x.shape
    return np.ascontiguousarray(x.reshape(K // 128, 128, N).transpose(1, 0, 2))


def _bfw(x):
    return _wrap(np.asarray(x, np.float32)).astype(BF16)


def _rope_tables(pos):
    # pos: [n] int -> cos/sin [64, n] f32 (row j = dim j angle tables)
    inv = (1.0 / (10000.0 ** (np.arange(64, dtype=np.float32) / 64.0))).astype(np.float32)
    ang = inv[:, None] * pos[None, :].astype(np.float32)
    return np.cos(ang).astype(np.float32), np.sin(ang).astype(np.float32)


def host_prep(inputs):
    """Compute index/label/weight arrays and per-core device inputs."""
    ii = np.asarray(inputs["input_ids"]).astype(np.int64)
    anch = np.asarray(inputs["anchor_positions"]).astype(np.int64)
    hs = np.asarray(inputs["hidden_states"], np.float32)
    lmw = np.asarray(inputs["lm_head_weight"], np.float32)
    nw = np.asarray(inputs["norm_weight"], np.float32)
    fc = np.asarray(inputs["fc_weight"], np.float32)
    emb = np.asarray(inputs["embed_table"], np.float32)
    wq = np.asarray(inputs["wq"], np.float32)
    wk = np.asarray(inputs["wk"], np.float32)
    wv = np.asarray(inputs["wv"], np.float32)
    wo = np.asarray(inputs["wo"], np.float32)

    offs = np.arange(BS, dtype=np.int64)
    pos_flat = (anch[:, :, None] + offs[None, None, :]).reshape(B, Q)
    in_bounds = pos_flat < S
    gidx = np.minimum(pos_flat, S - 1)
    all_tok = np.take_along_axis(ii, gidx, axis=1)
    pos_in_block = np.arange(Q) % BS
    is_anchor = pos_in_block == 0
    draft_ids = np.where(is_anchor[None, :], all_tok, MASK_ID)
    labels = np.where((~is_anchor)[None, :] & in_bounds, all_tok, -100)
    lbl = np.maximum(labels, 0)
    anc_q = anch[:, np.arange(Q) // BS]          # [B, Q] anchor per draft row

    # shared (batch-independent) tensors
    sc_q = 1.0 / math.sqrt(DH)
    lmn = (lmw * nw[None, :]).T                   # [D, V]
    shared = {
        "fcT": _bfw(fc.T),                        # [3072 -> D] kxm
        "wqTs": _bfw(wq.T * sc_q),
        "wkT": _bfw(wk.T),
        "wvT": _bfw(wv.T),
        "woT": _bfw(wo.T),
        "lmT": _wrap(np.clip(lmn * SW, -240.0, 240.0)).astype(F8NP),
        "ident_bf": np.eye(128, dtype=np.float32).astype(BF16),
        "ident_f32": np.eye(128, dtype=np.float32),
    }
    cosc, sinc = _rope_tables(np.arange(S))
    shared["cosc"], shared["sinc"] = cosc, sinc
    qi = np.arange(128)
    shared["dmask"] = np.where((qi[:, None] // BS) == (qi[None, :] // BS),
                               0.0, NEG).astype(np.float32)

    per_core = []
    for c in range(NCORES):
        b, r = c // RPG, c % RPG
        sl = slice(r * QL, (r + 1) * QL)
        hcat = hs[:, b].transpose(1, 0, 2).reshape(S, L * D)   # [S, 3072]
        embT = emb[draft_ids[b]].T                              # [D, Q]
        cosd, sind = _rope_tables(pos_flat[b, sl])
        kv = np.arange(S)
        # kv-major mask: [S rows, QL cols]
        mbT = np.where(kv[:, None] < anc_q[b, sl][None, :], 0.0, NEG).astype(np.float32)
        d = dict(shared)
        d.update({
            "hcatT": _bfw(hcat.T),                              # (128,24,2048)
            "embT": _bfw(embT[:, sl]),                          # (128,8,512)
            "lblT": _bfw(lmn[:, lbl[b, sl]]),                   # (128,8,512)
            "maskbT": _wrap(mbT),                               # (128,16,512) f32
            "cosd": cosd, "sind": sind,                         # [64,512]
        })
        per_core.append(d)

    meta = dict(labels=labels, lbl=lbl, pos_in_block=pos_in_block)
    return per_core, meta


def host_reduce(stats_list, meta):
    """stats_list: per-core [4, 512] f32 rows (maxexp, labdot, sumexp, rstd)."""
    labels = meta["labels"]
    pib = meta["pos_in_block"]
    decay = np.concatenate([np.zeros(1, np.float32),
                            np.exp(-(np.arange(1, BS, dtype=np.float32) - 1.0) / GAMMA)])
    w_all = decay[pib][None, :] * (labels != -100).astype(np.float32)

    num = 0.0
    den = 0.0
    ncorr = 0
    nvalid = int((labels != -100).sum())
    for c in range(NCORES):
        b, r = c // RPG, c % RPG
        st = stats_list[c]
        me, labd, sexp, rstd = st[0], st[1], st[2], st[3]
        nll = np.log(sexp) - rstd * labd
        w = w_all[b, r * QL:(r + 1) * QL]
        num += float((w * nll).sum())
        den += float(w.sum())
        valid = labels[b, r * QL:(r + 1) * QL] != -100
        m0 = np.log(np.maximum(me, 1e-30)) / rstd     # raw-logit units
        ncorr += int(((labd >= m0) & valid).sum())
    loss = np.float32(num / max(den, 1e-6))
    acc = np.float32(ncorr / max(nvalid, 1))
    return loss, acc


_PROG = None


def _rope(nc, pool, dst, src_ps, cos, sin, n):
    """dst[0:64] = x1*cos - x2*sin ; dst[64:128] = x1*sin + x2*cos.
    src_ps: [128, n] psum f32; cos/sin: [64, n] sbuf f32; dst: [128, n] bf16."""
    t1 = pool.tile([64, n], F32, tag="rope_t1")
    t2 = pool.tile([64, n], F32, tag="rope_t2")
    x1, x2 = src_ps[0:64, :], src_ps[64:128, :]
    nc.vector.tensor_mul(t1[:], x1, cos[:])
    nc.vector.tensor_mul(t2[:], x2, sin[:])
    nc.vector.tensor_sub(dst[0:64, :], t1[:], t2[:])
    nc.vector.tensor_mul(t1[:], x1, sin[:])
    nc.vector.tensor_mul(t2[:], x2, cos[:])
    nc.vector.tensor_add(dst[64:128, :], t1[:], t2[:])


def build_program():
    global _PROG
    if _PROG is not None:
        return _PROG
    import os
    phases = os.environ.get("DFLASH_PHASES", "123")
    lmparts = os.environ.get("DFLASH_LM", "abc")
    nc = bacc.Bacc(None, target_bir_lowering=False, debug=False)
    names = {}
    with tile.TileContext(nc) as tc:
        with tc.tile_pool(name="dram", bufs=1, space="DRAM") as dram:
            def din(name, shape, dt=BF):
                t = dram.tile(shape, dt, kind="ExternalInput", name=name)
                names[name] = t.name
                return t

            hcatT = din("hcatT", [128, 24, 2048])
            fcT = din("fcT", [128, 24, 1024])
            wqTs = din("wqTs", [128, 8, 1024])
            wkT = din("wkT", [128, 8, 1024])
            wvT = din("wvT", [128, 8, 1024])
            woT = din("woT", [128, 8, 1024])
            lmT = din("lmT", [128, 8, V], F8)
            embT = din("embT", [128, 8, QL])
            lblT = din("lblT", [128, 8, QL])
            maskbT = din("maskbT", [128, 16, QL], F32)
            cosc = din("cosc", [64, S], F32)
            sinc = din("sinc", [64, S], F32)
            cosd = din("cosd", [64, QL], F32)
            sind = din("sind", [64, QL], F32)
            dmask = din("dmask", [128, 128], F32)
            ident_bf = din("ident_bf", [128, 128])
            ident_f32 = din("ident_f32", [128, 128], F32)

            stats = dram.tile([4, QL], F32, kind="ExternalOutput", name="stats")
            names["stats"] = stats.name

            kT_d = dram.tile([128, 8, S + QL], BF, name="kT_scratch")
            v_d = dram.tile([128, KVW, 1024], BF, name="v_scratch")
            rstd_d = dram.tile([QL], F32, name="rstd_scratch")

            import contextlib
            with contextlib.ExitStack() as ctx:
                # psum pools shared across phases (<= 8 banks total)
                ps_big = ctx.enter_context(tc.tile_pool(name="ps_big", bufs=4, space="PSUM"))
                ps_tr = ctx.enter_context(tc.tile_pool(name="ps_tr", bufs=2, space="PSUM"))
                ps_row = ctx.enter_context(tc.tile_pool(name="ps_row", bufs=2, space="PSUM"))
                persist = ctx.enter_context(tc.tile_pool(name="persist", bufs=1))

                qTr = persist.tile([128, 8, QL], BF)        # roped q, feature-major
                embT_sb = persist.tile([128, 8, QL], BF)
                hbf = persist.tile([128, 8, QL], BF)
                h8 = persist.tile([128, 8, QL], F8)
                ones_bf = persist.tile([128, 1], BF)
                ones_f32 = persist.tile([128, 1], F32)
                ones_row = persist.tile([1, 128], BF)
                eps_t = persist.tile([1, 1], F32)
                nc.vector.memset(ones_bf[:], 1.0)
                nc.vector.memset(ones_f32[:], 1.0)
                nc.vector.memset(ones_row[:], 1.0)
                nc.vector.memset(eps_t[:], EPS)
                nc.sync.dma_start(out=embT_sb[:], in_=embT[:])

                # ---------- phase 1: draft projections + ctx K/V ----------
                if "1" in phases:
                  with tc.tile_pool(name="ph1", bufs=2) as ph1, \
                     tc.tile_pool(name="ph1w", bufs=1) as ph1w:
                    wq_sb = ph1w.tile([128, 8, 1024], BF)
                    wk_sb = ph1w.tile([128, 8, 1024], BF)
                    wv_sb = ph1w.tile([128, 8, 1024], BF)
                    cosd_sb = ph1w.tile([64, QL], F32)
                    sind_sb = ph1w.tile([64, QL], F32)
                    cosc_sb = ph1w.tile([64, S], F32)
                    sinc_sb = ph1w.tile([64, S], F32)
                    nc.sync.dma_start(out=wq_sb[:], in_=wqTs[:])
                    nc.sync.dma_start(out=wk_sb[:], in_=wkT[:])
                    nc.sync.dma_start(out=wv_sb[:], in_=wvT[:])
                    nc.sync.dma_start(out=cosd_sb[:], in_=cosd[:])
                    nc.sync.dma_start(out=sind_sb[:], in_=sind[:])
                    nc.sync.dma_start(out=cosc_sb[:], in_=cosc[:])
                    nc.sync.dma_start(out=sinc_sb[:], in_=sinc[:])

                    # draft q/k (feature-major, roped) and v (token-major)
                    for mb in range(8):
                        qp = ps_big.tile([128, QL], F32, tag="mm")
                        for kb in range(8):
                            nc.tensor.matmul(qp[:], wq_sb[:, kb, mb * 128:(mb + 1) * 128],
                                             embT_sb[:, kb, :], start=kb == 0, stop=kb == 7)
                        _rope(nc, ph1, qTr[:, mb, :], qp, cosd_sb, sind_sb, QL)
                    for mb in range(8):
                        kp = ps_big.tile([128, QL], F32, tag="mm")
                        for kb in range(8):
                            nc.tensor.matmul(kp[:], wk_sb[:, kb, mb * 128:(mb + 1) * 128],
                                             embT_sb[:, kb, :], start=kb == 0, stop=kb == 7)
                        kd_sb = ph1.tile([128, QL], BF, tag="kd")
                        _rope(nc, ph1, kd_sb[:], kp, cosd_sb, sind_sb, QL)
                        nc.sync.dma_start(out=kT_d[:, mb, S:S + QL], in_=kd_sb[:])
                    for sm in range(4):
                        for nn2 in range(2):
                            vp = ps_big.tile([128, 512], F32, tag="mm")
                            for kb in range(8):
                                nc.tensor.matmul(vp[:], embT_sb[:, kb, sm * 128:(sm + 1) * 128],
                                                 wv_sb[:, kb, nn2 * 512:(nn2 + 1) * 512],
                                                 start=kb == 0, stop=kb == 7)
                            vd_sb = ph1.tile([128, 512], BF, tag="vd")
                            nc.vector.tensor_copy(vd_sb[:], vp[:])
                            nc.sync.dma_start(out=v_d[:, 16 + sm, nn2 * 512:(nn2 + 1) * 512],
                                              in_=vd_sb[:])

                    # ctx chunks: ctxT -> kcT (roped) + vc
                    for sc in range(SCH):
                        ssl = slice(sc * 512, (sc + 1) * 512)
                        hc_sb = ph1.tile([128, 24, 512], BF, tag="hcat")
                        nc.sync.dma_start(out=hc_sb[:], in_=hcatT[:, :, ssl])
                        ctx_sb = ph1.tile([128, 8, 512], BF, tag="ctx")
                        for mb in range(8):
                            fcmb = ph1.tile([128, 24, 128], BF, tag="fcmb")
                            nc.sync.dma_start(out=fcmb[:], in_=fcT[:, :, mb * 128:(mb + 1) * 128])
                            cp = ps_big.tile([128, 512], F32, tag="mm")
                            for kb in range(24):
                                nc.tensor.matmul(cp[:], fcmb[:, kb, :],
                                                 hc_sb[:, kb, :], start=kb == 0, stop=kb == 23)
                            nc.vector.tensor_copy(ctx_sb[:, mb, :], cp[:])
                        for mb in range(8):
                            kp = ps_big.tile([128, 512], F32, tag="mm")
                            for kb in range(8):
                                nc.tensor.matmul(kp[:], wk_sb[:, kb, mb * 128:(mb + 1) * 128],
                                                 ctx_sb[:, kb, :], start=kb == 0, stop=kb == 7)
                            kc_sb = ph1.tile([128, 512], BF, tag="kc")
                            _rope(nc, ph1, kc_sb[:], kp, cosc_sb[:, ssl], sinc_sb[:, ssl], 512)
                            nc.sync.dma_start(out=kT_d[:, mb, ssl], in_=kc_sb[:])
                        for sm in range(4):
                            for nn2 in range(2):
                                vp = ps_big.tile([128, 512], F32, tag="mm")
                                for kb in range(8):
                                    nc.tensor.matmul(vp[:], ctx_sb[:, kb, sm * 128:(sm + 1) * 128],
                                                     wv_sb[:, kb, nn2 * 512:(nn2 + 1) * 512],
                                                     start=kb == 0, stop=kb == 7)
                                vc_sb = ph1.tile([128, 512], BF, tag="vc")
                                nc.vector.tensor_copy(vc_sb[:], vp[:])
                                nc.sync.dma_start(out=v_d[:, sc * 4 + sm, nn2 * 512:(nn2 + 1) * 512],
                                                  in_=vc_sb[:])

                # ---------- phase 2: attention (kv-major scores) ----------
                if "2" in phases:
                  with tc.tile_pool(name="ph2", bufs=2) as ph2, \
                     tc.tile_pool(name="ph2c", bufs=1) as ph2c, \
                     tc.tile_pool(name="ph2p", bufs=3) as ph2p:
                    maskT_sb = ph2c.tile([128, 16, QL], F32)
                    nc.sync.dma_start(out=maskT_sb[:], in_=maskbT[:])
                    dmask_sb = ph2c.tile([128, 128], F32)
                    nc.sync.dma_start(out=dmask_sb[:], in_=dmask[:])
                    wo_sb = ph2c.tile([128, 8, 1024], BF)
                    nc.sync.dma_start(out=wo_sb[:], in_=woT[:])
                    outT = ph2c.tile([128, 8, QL], BF)
                    h_sb = ph2c.tile([128, 8, QL], F32)

                    for h in range(H):
                        kh_sb = ph2.tile([128, S + QL], BF, tag="kh")
                        nc.sync.dma_start(out=kh_sb[:], in_=kT_d[:, h, :])
                        vh_sb = ph2.tile([128, KVW, 128], BF, tag="vh")
                        nc.sync.dma_start(out=vh_sb[:], in_=v_d[:, :, h * 128:(h + 1) * 128])
                        pT = ph2.tile([128, KVW, QL], BF, tag="pT")
                        nc.vector.memset(pT[:, 16:20, :], 0.0)
                        for t in range(16):
                            sp = ps_big.tile([128, QL], F32, tag="mm")
                            nc.tensor.matmul(sp[:], kh_sb[:, t * 128:(t + 1) * 128],
                                             qTr[:, h, :])
                            sm_sb = ph2p.tile([128, QL], F32, tag="smask")
                            nc.vector.tensor_add(sm_sb[:], sp[:], maskT_sb[:, t, :])
                            nc.scalar.activation(pT[:, t, :], sm_sb[:], ACTF.Exp)
                        for t in range(4):
                            qsl = slice(t * 128, (t + 1) * 128)
                            spd = ps_tr.tile([128, 128], F32, tag="small")
                            nc.tensor.matmul(spd[:], kh_sb[:, S + t * 128:S + (t + 1) * 128],
                                             qTr[:, h, qsl])
                            smd = ph2p.tile([128, 128], F32, tag="smaskd")
                            nc.vector.tensor_add(smd[:], spd[:], dmask_sb[:])
                            nc.scalar.activation(pT[:, 16 + t, qsl], smd[:], ACTF.Exp)

                        dp = ps_row.tile([1, QL], F32, tag="row")
                        for cc in range(KVW):
                            nc.tensor.matmul(dp[:], ones_bf[:], pT[:, cc, :],
                                             start=cc == 0, stop=cc == KVW - 1)
                        den_sb = ph2.tile([1, QL], BF, tag="den_sb")
                        with nc.allow_low_precision(reason="attn denom bf16 for bcast matmul"):
                            nc.vector.reciprocal(den_sb[:], dp[:])
                        den_ps = ps_big.tile([128, QL], F32, tag="mm")
                        nc.tensor.matmul(den_ps[:], ones_row[:], den_sb[:])

                        ap_ps = ps_big.tile([128, QL], F32, tag="mm")
                        for cc in range(KVW):
                            nc.tensor.matmul(ap_ps[:], vh_sb[:, cc, :], pT[:, cc, :],
                                             start=cc == 0, stop=cc == KVW - 1)
                        av_sb = ph2p.tile([128, QL], BF, tag="av")
                        nc.vector.tensor_copy(av_sb[:], ap_ps[:])
                        nc.vector.tensor_mul(outT[:, h, :], av_sb[:], den_ps[:])

                    # wo projection + residual + sumsq/rstd + fp8 cast
                    sqp = ps_row.tile([1, QL], F32, tag="row")
                    for mb in range(8):
                        op = ps_big.tile([128, QL], F32, tag="mm")
                        for kb in range(8):
                            nc.tensor.matmul(op[:], wo_sb[:, kb, mb * 128:(mb + 1) * 128],
                                             outT[:, kb, :], start=kb == 0, stop=kb == 7)
                        nc.vector.tensor_add(h_sb[:, mb, :], op[:], embT_sb[:, mb, :])
                        nc.vector.tensor_copy(hbf[:, mb, :], h_sb[:, mb, :])
                        nc.vector.tensor_scalar_mul(h8[:, mb, :], h_sb[:, mb, :], SH)
                        hsq = ph2.tile([128, QL], F32, tag="hsq")
                        nc.vector.tensor_mul(hsq[:], h_sb[:, mb, :], h_sb[:, mb, :])
                        nc.tensor.matmul(sqp[:], ones_f32[:], hsq[:],
                                         start=mb == 0, stop=mb == 7)
                    sq_sb = ph2.tile([1, QL], F32, tag="sq_sb")
                    nc.scalar.activation(sq_sb[:], sqp[:], ACTF.Sqrt,
                                         bias=eps_t[0:1, :], scale=1.0 / D)
                    rstd_sb = ph2.tile([1, QL], F32, tag="rstd_sb")
                    nc.vector.reciprocal(rstd_sb[:], sq_sb[:])
                    nc.sync.dma_start(out=rstd_d[:], in_=rstd_sb[0:1, :])
                    nc.sync.dma_start(out=stats[3:4, :], in_=rstd_sb[:])

                # ---------- phase 3: LM head (fp8 DoubleRow) ----------
                if "3" in phases:
                  with tc.tile_pool(name="ph3", bufs=2) as ph3, \
                     tc.tile_pool(name="ph3s", bufs=4) as ph3s, \
                     tc.tile_pool(name="ph3c", bufs=1) as ph3c, \
                     tc.tile_pool(name="ph3a", bufs=1) as ph3a:
                    lbl_sb = ph3c.tile([128, 8, QL], BF)
                    nc.sync.dma_start(out=lbl_sb[:], in_=lblT[:])
                    idf_sb = ph3c.tile([128, 128], F32)
                    nc.sync.dma_start(out=idf_sb[:], in_=ident_f32[:])
                    me = [ph3a.tile([128, 1], F32, name=f"me_{qt}") for qt in range(4)]
                    se = [ph3a.tile([128, 1], F32, name=f"se_{qt}") for qt in range(4)]
                    rstd_s = [ph3a.tile([128, 1], F32, name=f"rstd_s_{qt}") for qt in range(4)]
                    for qt in range(4):
                        nc.vector.memset(me[qt][:], 0.0)
                        nc.vector.memset(se[qt][:], 0.0)
                        rp = ph3s.tile([128, 1], F32, tag="rp")
                        nc.sync.dma_start(out=rp[:], in_=rstd_d[qt * 128:(qt + 1) * 128])
                        nc.vector.tensor_scalar_mul(rstd_s[qt][:], rp[:], 1.0 / SHW)

                    for vg in range(NVG):
                        ncol = min(VG, V - vg * VG)
                        lmt = ph3.tile([128, 8, VG], F8, tag="lmt")
                        nc.sync.dma_start(out=lmt[:, :, :ncol], in_=lmT[:, :, vg * VG:vg * VG + ncol])
                        for qt in range(4):
                            qsl = slice(qt * 128, (qt + 1) * 128)
                            nch = (ncol + 511) // 512
                            pss = [ps_big.tile([128, 512], F32, tag="mm", name=f"lp_{vg}_{qt}_{i}")
                                   for i in range(nch)]
                            for kb2 in range(4):
                                for cc in range(nch):
                                    w = min(512, ncol - cc * 512)
                                    nc.tensor.matmul(pss[cc][:, :w],
                                                     h8[:, 2 * kb2:2 * kb2 + 2, qsl],
                                                     lmt[:, 2 * kb2:2 * kb2 + 2, cc * 512:cc * 512 + w],
                                                     start=kb2 == 0, stop=kb2 == 3,
                                                     perf_mode=DR)
                            for cc in range(nch):
                                w = min(512, ncol - cc * 512)
                                lp = pss[cc]
                                if "b" in lmparts:
                                    ej = ph3s.tile([128, 512], BF, tag="ej")
                                    csum = ph3s.tile([128, 1], F32, tag="csum")
                                    nc.scalar.activation(ej[:, :w], lp[:, :w], ACTF.Exp,
                                                         scale=rstd_s[qt][:], accum_out=csum[:])
                                    nc.vector.tensor_add(se[qt][:], se[qt][:], csum[:])
                                    if "a" in lmparts:
                                        emax = ph3s.tile([128, 1], F32, tag="emax")
                                        nc.vector.tensor_reduce(emax[:], ej[:, :w], axis=AX.X, op=OP.max)
                                        nc.vector.tensor_max(me[qt][:], me[qt][:], emax[:])

                    for qt in range(4):
                        qsl = slice(qt * 128, (qt + 1) * 128)
                        if "c" in lmparts:
                            lpp = ps_tr.tile([128, 128], F32, tag="small")
                            for kb in range(8):
                                nc.tensor.matmul(lpp[:], hbf[:, kb, qsl], lbl_sb[:, kb, qsl],
                                                 start=kb == 0, stop=kb == 7)
                            junk = ph3s.tile([128, 128], F32, tag="junk")
                            labd = ph3s.tile([128, 1], F32, tag="labd")
                            nc.vector.tensor_mul(junk[:], lpp[:], idf_sb[:])
                            nc.vector.tensor_reduce(labd[:], junk[:], axis=AX.X, op=OP.add)
                            nc.sync.dma_start(out=stats[1:2, qsl], in_=labd[:])
                        if "a" in lmparts:
                            nc.sync.dma_start(out=stats[0:1, qsl], in_=me[qt][:])
                        if "b" in lmparts:
                            nc.sync.dma_start(out=stats[2:3, qsl], in_=se[qt][:])
    nc.compile()
    _PROG = (nc, names)
    return _PROG


def kernel(**inputs):
    import os
    from concourse.bass_utils import run_bass_kernel_spmd
    nc, names = build_program()
    per_core, meta = host_prep(inputs)
    in_maps = []
    for c in range(NCORES):
        m = {}
        for k, arr in per_core[c].items():
            if arr.dtype == BF16 or arr.dtype == F8NP:
                dt = arr.dtype
            else:
                dt = np.float32
            m[names[k]] = np.ascontiguousarray(arr, dtype=dt)
        in_maps.append(m)
    trace = bool(os.environ.get("DFLASH_TRACE"))
    if trace:
        try:
            import ntff_shim
            ntff_shim.install()
        except Exception:
            trace = False
    res = run_bass_kernel_spmd(nc, in_maps, list(range(NCORES)), trace=trace)
    kernel.last_exec_ns = res.exec_time_ns
    kernel.last_trace = getattr(res, "instructions_and_trace", None)
    kernel.last_profile_json = getattr(res, "profile_json", None)
    stats_list = [res.results[c][names["stats"]] for c in range(NCORES)]
    return host_reduce(stats_list, meta)


# revision 14
# speedup vs baseline: 1.8595x; 1.2166x over previous
"""DFlash draft-model loss/acc kernel for 8 Trainium2 NeuronCores.

Sharding: core c -> (batch b = c//4, query-quarter r = c%4).
Each core computes context features + K/V for its batch (bf16 matmuls,
fp32 accumulation), attention + LM head for its 512 draft rows over the
full vocab, and returns per-row stats (max exp, raw label logit,
scaled sum-exp, rstd). The host computes the weighted CE loss and
accuracy from the stats.

Phase 2 computes scores kv-major (no transposes); phase 3 runs the LM
head in fp8e4 DoubleRow mode (2x tensor throughput) with scale factors
SH/SW folded out via the exp scale and host-side rescaling.
"""
import math
import sys

sys.path.insert(0, "/opt/trn_rl_repo")

import numpy as np
import ml_dtypes

import concourse.bass as bass
import concourse.mybir as mybir
import concourse.tile as tile
from concourse import bacc

BF16 = ml_dtypes.bfloat16
F8NP = ml_dtypes.float8_e4m3
F32 = mybir.dt.float32
BF = mybir.dt.bfloat16
F8 = mybir.dt.float8e4
AX = mybir.AxisListType
OP = mybir.AluOpType
ACTF = mybir.ActivationFunctionType
DR = mybir.MatmulPerfMode.DoubleRow

L, B, S, D = 3, 2, 2048, 1024
H, DH = 8, 128
NA, BS = 128, 16
Q = NA * BS            # 2048 draft tokens per batch
V = 32000
MASK_ID = V - 1
GAMMA, EPS = 7.0, 1e-6
NCORES, RPG = 8, 4     # 2 batch groups x 4 row-quarters
QL = Q // RPG          # 512 local draft rows per core
SCH = S // 512         # 4 ctx chunks of 512
KVW = (S + QL) // 128  # 20 kv wrap-blocks (16 ctx + 4 draft)
NEG = -1.0e30
VG = 2048                          # vocab staging group (cols)
NVG = (V + VG - 1) // VG           # 16 staging groups
SH = 64.0                          # fp8 scale for hidden states
SW = 2048.0                        # fp8 scale for lm head weights
SHW = SH * SW


def _wrap(x):
    # [K, N] row-major -> (128, K//128, N): [p, kb, n] = x[kb*128 + p, n]
    K, N = x.shape
    return np.ascontiguousarray(x.reshape(K // 128, 128, N).transpose(1, 0, 2))


def _bfw(x):
    return _wrap(np.asarray(x, np.float32)).astype(BF16)


def _rope_tables(pos):
    # pos: [n] int -> cos/sin [64, n] f32 (row j = dim j angle tables)
    inv = (1.0 / (10000.0 ** (np.arange(64, dtype=np.float32) / 64.0))).astype(np.float32)
    ang = inv[:, None] * pos[None, :].astype(np.float32)
    return np.cos(ang).astype(np.float32), np.sin(ang).astype(np.float32)


def host_prep(inputs):
    """Compute index/label/weight arrays and per-core device inputs."""
    ii = np.asarray(inputs["input_ids"]).astype(np.int64)
    anch = np.asarray(inputs["anchor_positions"]).astype(np.int64)
    hs = np.asarray(inputs["hidden_states"], np.float32)
    lmw = np.asarray(inputs["lm_head_weight"], np.float32)
    nw = np.asarray(inputs["norm_weight"], np.float32)
    fc = np.asarray(inputs["fc_weight"], np.float32)
    emb = np.asarray(inputs["embed_table"], np.float32)
    wq = np.asarray(inputs["wq"], np.float32)
    wk = np.asarray(inputs["wk"], np.float32)
    wv = np.asarray(inputs["wv"], np.float32)
    wo = np.asarray(inputs["wo"], np.float32)

    offs = np.arange(BS, dtype=np.int64)
    pos_flat = (anch[:, :, None] + offs[None, None, :]).reshape(B, Q)
    in_bounds = pos_flat < S
    gidx = np.minimum(pos_flat, S - 1)
    all_tok = np.take_along_axis(ii, gidx, axis=1)
    pos_in_block = np.arange(Q) % BS
    is_anchor = pos_in_block == 0
    draft_ids = np.where(is_anchor[None, :], all_tok, MASK_ID)
    labels = np.where((~is_anchor)[None, :] & in_bounds, all_tok, -100)
    lbl = np.maximum(labels, 0)
    anc_q = anch[:, np.arange(Q) // BS]          # [B, Q] anchor per draft row

    # shared (batch-independent) tensors
    sc_q = 1.0 / math.sqrt(DH)
    lmn = (lmw * nw[None, :]).T                   # [D, V]
    shared = {
        "fcT": _bfw(fc.T),                        # [3072 -> D] kxm
        "wqTs": _bfw(wq.T * sc_q),
        "wkT": _bfw(wk.T),
        "wvT": _bfw(wv.T),
        "woT": _bfw(wo.T),
        "lmT": _wrap(np.clip(lmn * SW, -240.0, 240.0)).astype(F8NP),
        "ident_bf": np.eye(128, dtype=np.float32).astype(BF16),
        "ident_f32": np.eye(128, dtype=np.float32),
    }
    cosc_full, sinc_full = _rope_tables(np.arange(S))
    qi = np.arange(128)
    shared["dmask"] = np.where((qi[:, None] // BS) == (qi[None, :] // BS),
                               0.0, NEG).astype(np.float32)

    per_core = []
    for c in range(NCORES):
        b, r = c // RPG, c % RPG
        sl = slice(r * QL, (r + 1) * QL)
        hcat = hs[:, b].transpose(1, 0, 2).reshape(S, L * D)   # [S, 3072]
        embT = emb[draft_ids[b]].T                              # [D, Q]
        cosd, sind = _rope_tables(pos_flat[b, sl])
        kv = np.arange(S)
        # kv-major mask: [S rows, QL cols]
        mbT = np.where(kv[:, None] < anc_q[b, sl][None, :], 0.0, NEG).astype(np.float32)
        ssl = slice(r * 512, (r + 1) * 512)                     # ctx quarter for AG
        d = dict(shared)
        d.update({
            "hcatT": _bfw(hcat.T[:, ssl]),                      # (128,24,512) quarter
            "embT": _bfw(embT[:, sl]),                          # (128,8,512)
            "lblT": _bfw(lmn[:, lbl[b, sl]]),                   # (128,8,512)
            "maskbT": _wrap(mbT),                               # (128,16,512) f32
            "cosd": cosd, "sind": sind,                         # [64,512]
            "cosc": cosc_full[:, ssl], "sinc": sinc_full[:, ssl],
        })
        per_core.append(d)

    meta = dict(labels=labels, lbl=lbl, pos_in_block=pos_in_block)
    return per_core, meta


def host_reduce(stats_list, meta):
    """stats_list: per-core [4, 512] f32 rows (maxexp, labdot, sumexp, rstd)."""
    labels = meta["labels"]
    pib = meta["pos_in_block"]
    decay = np.concatenate([np.zeros(1, np.float32),
                            np.exp(-(np.arange(1, BS, dtype=np.float32) - 1.0) / GAMMA)])
    w_all = decay[pib][None, :] * (labels != -100).astype(np.float32)

    num = 0.0
    den = 0.0
    ncorr = 0
    nvalid = int((labels != -100).sum())
    for c in range(NCORES):
        b, r = c // RPG, c % RPG
        st = stats_list[c]
        me, labd, sexp, rstd = st[0], st[1], st[2], st[3]
        nll = np.log(sexp) - rstd * labd
        w = w_all[b, r * QL:(r + 1) * QL]
        num += float((w * nll).sum())
        den += float(w.sum())
        valid = labels[b, r * QL:(r + 1) * QL] != -100
        m0 = np.log(np.maximum(me, 1e-30)) / rstd     # raw-logit units
        ncorr += int(((labd >= m0) & valid).sum())
    loss = np.float32(num / max(den, 1e-6))
    acc = np.float32(ncorr / max(nvalid, 1))
    return loss, acc


_PROG = None


def _rope(nc, pool, dst, src_ps, cos, sin, n):
    """dst[0:64] = x1*cos - x2*sin ; dst[64:128] = x1*sin + x2*cos.
    src_ps: [128, n] psum f32; cos/sin: [64, n] sbuf f32; dst: [128, n] bf16."""
    t1 = pool.tile([64, n], F32, tag="rope_t1")
    t2 = pool.tile([64, n], F32, tag="rope_t2")
    x1, x2 = src_ps[0:64, :], src_ps[64:128, :]
    nc.vector.tensor_mul(t1[:], x1, cos[:])
    nc.vector.tensor_mul(t2[:], x2, sin[:])
    nc.vector.tensor_sub(dst[0:64, :], t1[:], t2[:])
    nc.vector.tensor_mul(t1[:], x1, sin[:])
    nc.vector.tensor_mul(t2[:], x2, cos[:])
    nc.vector.tensor_add(dst[64:128, :], t1[:], t2[:])


def build_program():
    global _PROG
    if _PROG is not None:
        return _PROG
    import os
    phases = os.environ.get("DFLASH_PHASES", "123")
    lmparts = os.environ.get("DFLASH_LM", "abc")
    nc = bacc.Bacc(None, target_bir_lowering=False, debug=False, num_devices=NCORES)
    names = {}
    RG = [[0, 1, 2, 3], [4, 5, 6, 7]]
    with tile.TileContext(nc) as tc:
        with tc.tile_pool(name="dram", bufs=1, space="DRAM") as dram:
            def din(name, shape, dt=BF):
                t = dram.tile(shape, dt, kind="ExternalInput", name=name)
                names[name] = t.name
                return t

            hcatT = din("hcatT", [128, 24, 512])
            fcT = din("fcT", [128, 24, 1024])
            wqTs = din("wqTs", [128, 8, 1024])
            wkT = din("wkT", [128, 8, 1024])
            wvT = din("wvT", [128, 8, 1024])
            woT = din("woT", [128, 8, 1024])
            lmT = din("lmT", [128, 8, V], F8)
            embT = din("embT", [128, 8, QL])
            lblT = din("lblT", [128, 8, QL])
            maskbT = din("maskbT", [128, 16, QL], F32)
            cosc = din("cosc", [64, 512], F32)
            sinc = din("sinc", [64, 512], F32)
            cosd = din("cosd", [64, QL], F32)
            sind = din("sind", [64, QL], F32)
            dmask = din("dmask", [128, 128], F32)
            ident_bf = din("ident_bf", [128, 128])
            ident_f32 = din("ident_f32", [128, 128], F32)

            stats = dram.tile([4, QL], F32, kind="ExternalOutput", name="stats")
            names["stats"] = stats.name

            kpart = dram.tile([128, 8, 512], BF, name="kpart")
            vpart = dram.tile([128, 4, 1024], BF, name="vpart")
            kall = dram.tile([4, 128, 8, 512], BF, name="kall")
            vall = dram.tile([4, 128, 4, 1024], BF, name="vall")
            kdraft_d = dram.tile([128, 8, QL], BF, name="kdraft")
            vdraft_d = dram.tile([128, 4, 1024], BF, name="vdraft")
            rstd_d = dram.tile([QL], F32, name="rstd_scratch")

            import contextlib
            with contextlib.ExitStack() as ctx:
                # psum pools shared across phases (<= 8 banks total)
                ps_big = ctx.enter_context(tc.tile_pool(name="ps_big", bufs=4, space="PSUM"))
                ps_tr = ctx.enter_context(tc.tile_pool(name="ps_tr", bufs=2, space="PSUM"))
                ps_row = ctx.enter_context(tc.tile_pool(name="ps_row", bufs=2, space="PSUM"))
                persist = ctx.enter_context(tc.tile_pool(name="persist", bufs=1))

                qTr = persist.tile([128, 8, QL], BF)        # roped q, feature-major
                embT_sb = persist.tile([128, 8, QL], BF)
                hbf = persist.tile([128, 8, QL], BF)
                h8 = persist.tile([128, 8, QL], F8)
                ones_bf = persist.tile([128, 1], BF)
                ones_f32 = persist.tile([128, 1], F32)
                ones_row = persist.tile([1, 128], BF)
                eps_t = persist.tile([1, 1], F32)
                nc.vector.memset(ones_bf[:], 1.0)
                nc.vector.memset(ones_f32[:], 1.0)
                nc.vector.memset(ones_row[:], 1.0)
                nc.vector.memset(eps_t[:], EPS)
                nc.sync.dma_start(out=embT_sb[:], in_=embT[:])

                # ---------- phase 1: draft projections + ctx K/V ----------
                if "1" in phases:
                  with tc.tile_pool(name="ph1", bufs=2) as ph1, \
                     tc.tile_pool(name="ph1w", bufs=1) as ph1w:
                    wq_sb = ph1w.tile([128, 8, 1024], BF)
                    wk_sb = ph1w.tile([128, 8, 1024], BF)
                    wv_sb = ph1w.tile([128, 8, 1024], BF)
                    cosd_sb = ph1w.tile([64, QL], F32)
                    sind_sb = ph1w.tile([64, QL], F32)
                    cosc_sb = ph1w.tile([64, 512], F32)
                    sinc_sb = ph1w.tile([64, 512], F32)
                    nc.sync.dma_start(out=wq_sb[:], in_=wqTs[:])
                    nc.sync.dma_start(out=wk_sb[:], in_=wkT[:])
                    nc.sync.dma_start(out=wv_sb[:], in_=wvT[:])
                    nc.sync.dma_start(out=cosd_sb[:], in_=cosd[:])
                    nc.sync.dma_start(out=sind_sb[:], in_=sind[:])
                    nc.sync.dma_start(out=cosc_sb[:], in_=cosc[:])
                    nc.sync.dma_start(out=sinc_sb[:], in_=sinc[:])

                    # local ctx quarter: ctxT -> kcT (roped) + vc, then AllGather
                    hc_sb = ph1.tile([128, 24, 512], BF, tag="hcat")
                    nc.sync.dma_start(out=hc_sb[:], in_=hcatT[:])
                    ctx_sb = ph1.tile([128, 8, 512], BF, tag="ctx")
                    for mb in range(8):
                        fcmb = ph1.tile([128, 24, 128], BF, tag="fcmb")
                        nc.sync.dma_start(out=fcmb[:], in_=fcT[:, :, mb * 128:(mb + 1) * 128])
                        cp = ps_big.tile([128, 512], F32, tag="mm")
                        for kb in range(24):
                            nc.tensor.matmul(cp[:], fcmb[:, kb, :],
                                             hc_sb[:, kb, :], start=kb == 0, stop=kb == 23)
                        nc.vector.tensor_copy(ctx_sb[:, mb, :], cp[:])
                    for mb in range(8):
                        kp = ps_big.tile([128, 512], F32, tag="mm")
                        for kb in range(8):
                            nc.tensor.matmul(kp[:], wk_sb[:, kb, mb * 128:(mb + 1) * 128],
                                             ctx_sb[:, kb, :], start=kb == 0, stop=kb == 7)
                        kc_sb = ph1.tile([128, 512], BF, tag="kc")
                        _rope(nc, ph1, kc_sb[:], kp, cosc_sb, sinc_sb, 512)
                        nc.sync.dma_start(out=kpart[:, mb, :], in_=kc_sb[:])
                    for sm in range(4):
                        for nn2 in range(2):
                            vp = ps_big.tile([128, 512], F32, tag="mm")
                            for kb in range(8):
                                nc.tensor.matmul(vp[:], ctx_sb[:, kb, sm * 128:(sm + 1) * 128],
                                                 wv_sb[:, kb, nn2 * 512:(nn2 + 1) * 512],
                                                 start=kb == 0, stop=kb == 7)
                            vc_sb = ph1.tile([128, 512], BF, tag="vc")
                            nc.vector.tensor_copy(vc_sb[:], vp[:])
                            nc.sync.dma_start(out=vpart[:, sm, nn2 * 512:(nn2 + 1) * 512],
                                              in_=vc_sb[:])
                    nc.gpsimd.collective_compute(
                        "AllGather", OP.bypass, replica_groups=RG,
                        ins=[kpart[:].opt()], outs=[kall[:].opt()])
                    nc.gpsimd.collective_compute(
                        "AllGather", OP.bypass, replica_groups=RG,
                        ins=[vpart[:].opt()], outs=[vall[:].opt()])

                    # draft q/k (feature-major, roped) and v (token-major)
                    for mb in range(8):
                        qp = ps_big.tile([128, QL], F32, tag="mm")
                        for kb in range(8):
                            nc.tensor.matmul(qp[:], wq_sb[:, kb, mb * 128:(mb + 1) * 128],
                                             embT_sb[:, kb, :], start=kb == 0, stop=kb == 7)
                        _rope(nc, ph1, qTr[:, mb, :], qp, cosd_sb, sind_sb, QL)
                    for mb in range(8):
                        kp = ps_big.tile([128, QL], F32, tag="mm")
                        for kb in range(8):
                            nc.tensor.matmul(kp[:], wk_sb[:, kb, mb * 128:(mb + 1) * 128],
                                             embT_sb[:, kb, :], start=kb == 0, stop=kb == 7)
                        kd_sb = ph1.tile([128, QL], BF, tag="kd")
                        _rope(nc, ph1, kd_sb[:], kp, cosd_sb, sind_sb, QL)
                        nc.sync.dma_start(out=kdraft_d[:, mb, :], in_=kd_sb[:])
                    for sm in range(4):
                        for nn2 in range(2):
                            vp = ps_big.tile([128, 512], F32, tag="mm")
                            for kb in range(8):
                                nc.tensor.matmul(vp[:], embT_sb[:, kb, sm * 128:(sm + 1) * 128],
                                                 wv_sb[:, kb, nn2 * 512:(nn2 + 1) * 512],
                                                 start=kb == 0, stop=kb == 7)
                            vd_sb = ph1.tile([128, 512], BF, tag="vd")
                            nc.vector.tensor_copy(vd_sb[:], vp[:])
                            nc.sync.dma_start(out=vdraft_d[:, sm, nn2 * 512:(nn2 + 1) * 512],
                                              in_=vd_sb[:])

                # ---------- phase 2: attention (kv-major scores) ----------
                if "2" in phases:
                  with tc.tile_pool(name="ph2", bufs=2) as ph2, \
                     tc.tile_pool(name="ph2c", bufs=1) as ph2c, \
                     tc.tile_pool(name="ph2p", bufs=3) as ph2p:
                    maskT_sb = ph2c.tile([128, 16, QL], F32)
                    nc.sync.dma_start(out=maskT_sb[:], in_=maskbT[:])
                    dmask_sb = ph2c.tile([128, 128], F32)
                    nc.sync.dma_start(out=dmask_sb[:], in_=dmask[:])
                    wo_sb = ph2c.tile([128, 8, 1024], BF)
                    nc.sync.dma_start(out=wo_sb[:], in_=woT[:])
                    outT = ph2c.tile([128, 8, QL], BF)
                    h_sb = ph2c.tile([128, 8, QL], F32)

                    for h in range(H):
                        kh_sb = ph2.tile([128, S + QL], BF, tag="kh")
                        for rk in range(4):
                            nc.sync.dma_start(out=kh_sb[:, rk * 512:(rk + 1) * 512],
                                              in_=kall[rk, :, h, :])
                        nc.sync.dma_start(out=kh_sb[:, S:S + QL], in_=kdraft_d[:, h, :])
                        vh_sb = ph2.tile([128, KVW, 128], BF, tag="vh")
                        for rk in range(4):
                            nc.sync.dma_start(out=vh_sb[:, rk * 4:rk * 4 + 4, :],
                                              in_=vall[rk, :, :, h * 128:(h + 1) * 128])
                        nc.sync.dma_start(out=vh_sb[:, 16:20, :],
                                          in_=vdraft_d[:, :, h * 128:(h + 1) * 128])
                        pT = ph2.tile([128, KVW, QL], BF, tag="pT")
                        nc.vector.memset(pT[:, 16:20, :], 0.0)
                        for t in range(16):
                            sp = ps_big.tile([128, QL], F32, tag="mm")
                            nc.tensor.matmul(sp[:], kh_sb[:, t * 128:(t + 1) * 128],
                                             qTr[:, h, :])
                            sm_sb = ph2p.tile([128, QL], F32, tag="smask")
                            nc.vector.tensor_add(sm_sb[:], sp[:], maskT_sb[:, t, :])
                            nc.scalar.activation(pT[:, t, :], sm_sb[:], ACTF.Exp)
                        for t in range(4):
                            qsl = slice(t * 128, (t + 1) * 128)
                            spd = ps_tr.tile([128, 128], F32, tag="small")
                            nc.tensor.matmul(spd[:], kh_sb[:, S + t * 128:S + (t + 1) * 128],
                                             qTr[:, h, qsl])
                            smd = ph2p.tile([128, 128], F32, tag="smaskd")
                            nc.vector.tensor_add(smd[:], spd[:], dmask_sb[:])
                            nc.scalar.activation(pT[:, 16 + t, qsl], smd[:], ACTF.Exp)

                        dp = ps_row.tile([1, QL], F32, tag="row")
                        for cc in range(KVW):
                            nc.tensor.matmul(dp[:], ones_bf[:], pT[:, cc, :],
                                             start=cc == 0, stop=cc == KVW - 1)
                        den_sb = ph2.tile([1, QL], BF, tag="den_sb")
                        with nc.allow_low_precision(reason="attn denom bf16 for bcast matmul"):
                            nc.vector.reciprocal(den_sb[:], dp[:])
                        den_ps = ps_big.tile([128, QL], F32, tag="mm")
                        nc.tensor.matmul(den_ps[:], ones_row[:], den_sb[:])

                        ap_ps = ps_big.tile([128, QL], F32, tag="mm")
                        for cc in range(KVW):
                            nc.tensor.matmul(ap_ps[:], vh_sb[:, cc, :], pT[:, cc, :],
                                             start=cc == 0, stop=cc == KVW - 1)
                        av_sb = ph2p.tile([128, QL], BF, tag="av")
                        nc.vector.tensor_copy(av_sb[:], ap_ps[:])
                        nc.vector.tensor_mul(outT[:, h, :], av_sb[:], den_ps[:])

                    # wo projection + residual + sumsq/rstd + fp8 cast
                    sqp = ps_row.tile([1, QL], F32, tag="row")
                    for mb in range(8):
                        op = ps_big.tile([128, QL], F32, tag="mm")
                        for kb in range(8):
                            nc.tensor.matmul(op[:], wo_sb[:, kb, mb * 128:(mb + 1) * 128],
                                             outT[:, kb, :], start=kb == 0, stop=kb == 7)
                        nc.vector.tensor_add(h_sb[:, mb, :], op[:], embT_sb[:, mb, :])
                        nc.vector.tensor_copy(hbf[:, mb, :], h_sb[:, mb, :])
                        nc.vector.tensor_scalar_mul(h8[:, mb, :], h_sb[:, mb, :], SH)
                        hsq = ph2.tile([128, QL], F32, tag="hsq")
                        nc.vector.tensor_mul(hsq[:], h_sb[:, mb, :], h_sb[:, mb, :])
                        nc.tensor.matmul(sqp[:], ones_f32[:], hsq[:],
                                         start=mb == 0, stop=mb == 7)
                    sq_sb = ph2.tile([1, QL], F32, tag="sq_sb")
                    nc.scalar.activation(sq_sb[:], sqp[:], ACTF.Sqrt,
                                         bias=eps_t[0:1, :], scale=1.0 / D)
                    rstd_sb = ph2.tile([1, QL], F32, tag="rstd_sb")
                    nc.vector.reciprocal(rstd_sb[:], sq_sb[:])
                    nc.sync.dma_start(out=rstd_d[:], in_=rstd_sb[0:1, :])
                    nc.sync.dma_start(out=stats[3:4, :], in_=rstd_sb[:])

                # ---------- phase 3: LM head (fp8 DoubleRow) ----------
                if "3" in phases:
                  with tc.tile_pool(name="ph3", bufs=2) as ph3, \
                     tc.tile_pool(name="ph3s", bufs=4) as ph3s, \
                     tc.tile_pool(name="ph3c", bufs=1) as ph3c, \
                     tc.tile_pool(name="ph3a", bufs=1) as ph3a:
                    lbl_sb = ph3c.tile([128, 8, QL], BF)
                    nc.sync.dma_start(out=lbl_sb[:], in_=lblT[:])
                    idf_sb = ph3c.tile([128, 128], F32)
                    nc.sync.dma_start(out=idf_sb[:], in_=ident_f32[:])
                    me = [ph3a.tile([128, 1], F32, name=f"me_{qt}") for qt in range(4)]
                    se = [ph3a.tile([128, 1], F32, name=f"se_{qt}") for qt in range(4)]
                    rstd_s = [ph3a.tile([128, 1], F32, name=f"rstd_s_{qt}") for qt in range(4)]
                    for qt in range(4):
                        nc.vector.memset(me[qt][:], 0.0)
                        nc.vector.memset(se[qt][:], 0.0)
                        rp = ph3s.tile([128, 1], F32, tag="rp")
                        nc.sync.dma_start(out=rp[:], in_=rstd_d[qt * 128:(qt + 1) * 128])
                        nc.vector.tensor_scalar_mul(rstd_s[qt][:], rp[:], 1.0 / SHW)

                    for vg in range(NVG):
                        ncol = min(VG, V - vg * VG)
                        lmt = ph3.tile([128, 8, VG], F8, tag="lmt")
                        nc.sync.dma_start(out=lmt[:, :, :ncol], in_=lmT[:, :, vg * VG:vg * VG + ncol])
                        for qt in range(4):
                            qsl = slice(qt * 128, (qt + 1) * 128)
                            nch = (ncol + 511) // 512
                            pss = [ps_big.tile([128, 512], F32, tag="mm", name=f"lp_{vg}_{qt}_{i}")
                                   for i in range(nch)]
                            for kb2 in range(4):
                                for cc in range(nch):
                                    w = min(512, ncol - cc * 512)
                                    nc.tensor.matmul(pss[cc][:, :w],
                                                     h8[:, 2 * kb2:2 * kb2 + 2, qsl],
                                                     lmt[:, 2 * kb2:2 * kb2 + 2, cc * 512:cc * 512 + w],
                                                     start=kb2 == 0, stop=kb2 == 3,
                                                     perf_mode=DR)
                            for cc in range(nch):
                                w = min(512, ncol - cc * 512)
                                lp = pss[cc]
                                if "b" in lmparts:
                                    ej = ph3s.tile([128, 512], BF, tag="ej")
                                    csum = ph3s.tile([128, 1], F32, tag="csum")
                                    nc.scalar.activation(ej[:, :w], lp[:, :w], ACTF.Exp,
                                                         scale=rstd_s[qt][:], accum_out=csum[:])
                                    nc.vector.tensor_add(se[qt][:], se[qt][:], csum[:])
                                    if "a" in lmparts:
                                        emax = ph3s.tile([128, 1], F32, tag="emax")
                                        nc.vector.tensor_reduce(emax[:], ej[:, :w], axis=AX.X, op=OP.max)
                                        nc.vector.tensor_max(me[qt][:], me[qt][:], emax[:])

                    for qt in range(4):
                        qsl = slice(qt * 128, (qt + 1) * 128)
                        if "c" in lmparts:
                            lpp = ps_tr.tile([128, 128], F32, tag="small")
                            for kb in range(8):
                                nc.tensor.matmul(lpp[:], hbf[:, kb, qsl], lbl_sb[:, kb, qsl],
                                                 start=kb == 0, stop=kb == 7)
                            junk = ph3s.tile([128, 128], F32, tag="junk")
                            labd = ph3s.tile([128, 1], F32, tag="labd")
                            nc.vector.tensor_mul(junk[:], lpp[:], idf_sb[:])
                            nc.vector.tensor_reduce(labd[:], junk[:], axis=AX.X, op=OP.add)
                            nc.sync.dma_start(out=stats[1:2, qsl], in_=labd[:])
                        if "a" in lmparts:
                            nc.sync.dma_start(out=stats[0:1, qsl], in_=me[qt][:])
                        if "b" in lmparts:
                            nc.sync.dma_start(out=stats[2:3, qsl], in_=se[qt][:])
    nc.compile()
    _PROG = (nc, names)
    return _PROG


def kernel(**inputs):
    import os
    from concourse.bass_utils import run_bass_kernel_spmd
    nc, names = build_program()
    per_core, meta = host_prep(inputs)
    in_maps = []
    for c in range(NCORES):
        m = {}
        for k, arr in per_core[c].items():
            if arr.dtype == BF16 or arr.dtype == F8NP:
                dt = arr.dtype
            else:
                dt = np.float32
            m[names[k]] = np.ascontiguousarray(arr, dtype=dt)
        in_maps.append(m)
    trace = bool(os.environ.get("DFLASH_TRACE"))
    if trace:
        try:
            import ntff_shim
            ntff_shim.install()
        except Exception:
            trace = False
    res = run_bass_kernel_spmd(nc, in_maps, list(range(NCORES)), trace=trace)
    kernel.last_exec_ns = res.exec_time_ns
    kernel.last_trace = getattr(res, "instructions_and_trace", None)
    kernel.last_profile_json = getattr(res, "profile_json", None)
    stats_list = [res.results[c][names["stats"]] for c in range(NCORES)]
    return host_reduce(stats_list, meta)
